# revision 2
# baseline (speedup 1.0000x reference)
"""Trainium2 Bass kernel for nn_Attn: attn = softmax(outputs @ W.T @ wv + b @ wv).

Math: energy[s] = dot(wv, W @ outputs[s] + b) = outputs[s] . (wv @ W) + const.
The const (wv . b) cancels in softmax, and W collapses into v = wv @ W, so the
heavy work is a memory-bound [65536, 1024] @ [1024] matvec (256 MB of reads).

Sharding: outputs split along seq across 8 cores (8192 rows / 32 MB each).
Each core computes v = wv @ W on the PE (W replicated), broadcasts v across
partitions, streams its shard through a fused multiply-reduce on the vector
engine, AllGathers the 65536 energies (256 KB), and does the global softmax
on-chip. Every core writes the full [65536] softmax; the host takes core 0's.
"""

import sys

if "/opt/trn_rl_repo" not in sys.path:
    sys.path.insert(0, "/opt/trn_rl_repo")

import numpy as np

import concourse.bacc as bacc
import concourse.bass_isa as bass_isa
import concourse.mybir as mybir
import concourse.tile as tile
from concourse.bass_utils import run_bass_kernel_spmd

N_CORES = 8
SEQ = 65536
H2 = 1024
LOCAL = SEQ // N_CORES          # 8192 rows per core
ROWS_PER_PART = LOCAL // 128    # 64 seq rows per SBUF partition
TILE_R = 4                      # seq rows per partition per data tile (2 MB tiles)
N_TILES = ROWS_PER_PART // TILE_R

FP32 = mybir.dt.float32

_nc_cache = {}


def _build_nc():
    nc = bacc.Bacc("TRN2", target_bir_lowering=False)
    x = nc.dram_tensor("x", [LOCAL, H2], FP32, kind="ExternalInput")
    W = nc.dram_tensor("W", [H2, H2], FP32, kind="ExternalInput")
    wv = nc.dram_tensor("wv", [1, H2], FP32, kind="ExternalInput")
    out = nc.dram_tensor("out", [SEQ], FP32, kind="ExternalOutput")

    with tile.TileContext(nc) as tc:
        with (
            tc.tile_pool(name="singles", bufs=1) as singles,
            tc.tile_pool(name="wpool", bufs=2) as wpool,
            tc.tile_pool(name="data", bufs=4) as data,
            tc.tile_pool(name="scratch", bufs=4) as scratch,
            tc.tile_pool(name="psum", bufs=2, space="PSUM") as psum,
            tc.tile_pool(name="dram", bufs=1, space="DRAM") as dram,
        ):
            # ---- v = wv @ W on the PE; lhsT chunks are wv with o on partitions ----
            wv_sb = singles.tile([128, 8], FP32)
            nc.sync.dma_start(
                out=wv_sb[:], in_=wv[:].rearrange("1 (j p) -> p j", p=128)
            )
            psum_v0 = psum.tile([1, 512], FP32, tag="psv0")
            psum_v1 = psum.tile([1, 512], FP32, tag="psv1")
            for j in range(8):
                Wt = wpool.tile([128, H2], FP32)
                nc.sync.dma_start(out=Wt[:], in_=W[128 * j : 128 * (j + 1), :])
                nc.tensor.matmul(
                    psum_v0[:], wv_sb[:, j : j + 1], Wt[:, 0:512],
                    start=(j == 0), stop=(j == 7),
                )
                nc.tensor.matmul(
                    psum_v1[:], wv_sb[:, j : j + 1], Wt[:, 512:1024],
                    start=(j == 0), stop=(j == 7),
                )
            v_sb = singles.tile([1, H2], FP32)
            nc.vector.tensor_copy(v_sb[:, 0:512], psum_v0[:])
            nc.vector.tensor_copy(v_sb[:, 512:1024], psum_v1[:])
            v_rep = singles.tile([128, H2], FP32)
            nc.gpsimd.partition_broadcast(v_rep[:], v_sb[:], channels=128)

            # ---- energies[s] = x[s] . v via fused multiply-reduce on DVE ----
            # Layout: partition p owns seq rows [64p, 64p+64); energies[p, c] is
            # local row 64p + c, so the DRAM store is contiguous per partition.
            energies = singles.tile([128, ROWS_PER_PART], FP32)
            x_r = x.rearrange("(p c) h -> p c h", p=128)
            for t in range(N_TILES):
                xt = data.tile([128, TILE_R, H2], FP32)
                nc.sync.dma_start(
                    out=xt[:], in_=x_r[:, TILE_R * t : TILE_R * (t + 1), :]
                )
                for r in range(TILE_R):
                    c = TILE_R * t + r
                    sc = scratch.tile([128, H2], FP32)
                    nc.vector.scalar_tensor_tensor(
                        out=sc[:],
                        in0=xt[:, r, :],
                        scalar=1.0,
                        in1=v_rep[:],
                        op0=mybir.AluOpType.bypass,
                        op1=mybir.AluOpType.mult,
                        accum_out=energies[:, c : c + 1],
                    )

            # ---- AllGather the 65536 energies (32 KB per rank) ----
            ag_in = dram.tile([LOCAL], FP32)
            ag_out = dram.tile([SEQ], FP32, addr_space="Shared")
            nc.sync.dma_start(
                out=ag_in[:].rearrange("(p c) -> p c", p=128), in_=energies[:]
            )
            nc.gpsimd.collective_compute(
                "AllGather",
                mybir.AluOpType.bypass,
                replica_groups=[list(range(N_CORES))],
                ins=[ag_in.opt()],
                outs=[ag_out.opt()],
            )

            # ---- global softmax over all 65536 energies ----
            cols = SEQ // 128  # 512
            eng_all = singles.tile([128, cols], FP32)
            nc.sync.dma_start(
                out=eng_all[:], in_=ag_out[:].rearrange("(p j) -> p j", p=128)
            )
            pmax = singles.tile([128, 1], FP32)
            nc.vector.tensor_reduce(
                out=pmax[:], in_=eng_all[:],
                axis=mybir.AxisListType.X, op=mybir.AluOpType.max,
            )
            gmax = singles.tile([128, 1], FP32)
            nc.gpsimd.partition_all_reduce(
                gmax[:], pmax[:], channels=128, reduce_op=bass_isa.ReduceOp.max
            )
            ngmax = singles.tile([128, 1], FP32)
            nc.scalar.mul(ngmax[:], gmax[:], -1.0)
            eexp = singles.tile([128, cols], FP32)
            sumexp = singles.tile([128, 1], FP32)
            nc.scalar.activation(
                eexp[:], eng_all[:], mybir.ActivationFunctionType.Exp,
                bias=ngmax[:], scale=1.0, accum_out=sumexp[:],
            )
            gsum = singles.tile([128, 1], FP32)
            nc.gpsimd.partition_all_reduce(
                gsum[:], sumexp[:], channels=128, reduce_op=bass_isa.ReduceOp.add
            )
            inv = singles.tile([128, 1], FP32)
            nc.vector.reciprocal(inv[:], gsum[:])
            outt = singles.tile([128, cols], FP32)
            nc.vector.tensor_scalar_mul(outt[:], eexp[:], inv[:])
            nc.sync.dma_start(
                out=out[:].rearrange("(p j) -> p j", p=128), in_=outt[:]
            )

    nc.compile()
    return nc


def _get_nc():
    if "nc" not in _nc_cache:
        _nc_cache["nc"] = _build_nc()
    return _nc_cache["nc"]


def run(outputs, W, b, weight_vec, trace=False):
    del b  # dot(wv, b) is a constant energy offset; softmax is shift-invariant
    nc = _get_nc()
    outputs = np.ascontiguousarray(outputs, dtype=np.float32)
    W = np.ascontiguousarray(W, dtype=np.float32)
    wv = np.ascontiguousarray(weight_vec, dtype=np.float32).reshape(1, H2)
    in_maps = [
        {"x": outputs[c * LOCAL : (c + 1) * LOCAL], "W": W, "wv": wv}
        for c in range(N_CORES)
    ]
    res = run_bass_kernel_spmd(nc, in_maps, list(range(N_CORES)), trace=trace)
    attn = res.results[0]["out"].reshape(1, 1, SEQ)
    return attn, res


def kernel(outputs, W, b, weight_vec):
    attn, _ = run(outputs, W, b, weight_vec)
    return attn


# revision 20
# speedup vs baseline: 1.0605x; 1.0605x over previous
"""Trainium2 Bass kernel for nn_Attn: attn = softmax(outputs @ W.T @ wv + b @ wv).

Math: energy[s] = dot(wv, W @ outputs[s] + b) = outputs[s] . (wv @ W) + const.
The const (wv . b) cancels in softmax, and W collapses into v = wv @ W, so the
heavy work is a memory-bound [65536, 1024] @ [1024] matvec (256 MB of reads).

Sharding: outputs split along seq across 8 cores (8192 rows / 32 MB each).
Each core computes v = wv @ W on the PE (W replicated), broadcasts v across
partitions, streams its shard through a fused multiply-reduce on the vector
engine, AllGathers the 65536 energies (256 KB), and does the global softmax
on-chip. Every core writes the full [65536] softmax; the host takes core 0's.
"""

import sys

if "/opt/trn_rl_repo" not in sys.path:
    sys.path.insert(0, "/opt/trn_rl_repo")

import numpy as np

import concourse.bacc as bacc
from concourse.masks import make_identity
import concourse.bass_isa as bass_isa
import concourse.mybir as mybir
import concourse.tile as tile
from concourse.bass_utils import run_bass_kernel_spmd

N_CORES = 8
SEQ = 65536
H2 = 1024
LOCAL = SEQ // N_CORES          # 8192 rows per core
ROWS_PER_PART = LOCAL // 128    # 64 seq rows per SBUF partition
TILE_R = 4                      # seq rows per partition per data tile (2 MB tiles)
N_TILES = ROWS_PER_PART // TILE_R

FP32 = mybir.dt.float32

_nc_cache = {}


def _build_nc(work_mult=1, n_reps=1, mode="full", data_bufs=12, tile_r=2,
              scratch_bufs=4, dma_eng="sync", act8=5):
    TILE_R = tile_r
    N_TILES = ROWS_PER_PART // TILE_R
    nc = bacc.Bacc("TRN2", target_bir_lowering=False)
    x = nc.dram_tensor("x", [LOCAL, H2], FP32, kind="ExternalInput")
    W = nc.dram_tensor("W", [H2, H2], FP32, kind="ExternalInput")
    wv = nc.dram_tensor("wv", [1, H2], FP32, kind="ExternalInput")
    out = nc.dram_tensor("out", [LOCAL], FP32, kind="ExternalOutput")

    with tile.TileContext(nc) as tc:
        with (
            tc.tile_pool(name="singles", bufs=1) as singles,
            tc.tile_pool(name="wpool", bufs=2) as wpool,
            tc.tile_pool(name="data", bufs=data_bufs) as data,
            tc.tile_pool(name="scratch", bufs=scratch_bufs) as scratch,
            tc.tile_pool(name="epool", bufs=2) as epool,
            tc.tile_pool(name="psum", bufs=1, space="PSUM") as psum,
            tc.tile_pool(name="dram", bufs=1, space="DRAM") as dram,
        ):
            # ---- v = wv @ W on the PE; lhsT chunks are wv with o on partitions ----
            wv_sb = singles.tile([128, 8], FP32)
            nc.sync.dma_start(
                out=wv_sb[:], in_=wv[:].rearrange("1 (j p) -> p j", p=128)
            )
            psum_v0 = psum.tile([1, 512], FP32, tag="psv0")
            psum_v1 = psum.tile([1, 512], FP32, tag="psv1")
            for j in range(8):
                Wt = wpool.tile([128, H2], FP32)
                nc.sync.dma_start(out=Wt[:], in_=W[128 * j : 128 * (j + 1), :])
                nc.tensor.matmul(
                    psum_v0[:], wv_sb[:, j : j + 1], Wt[:, 0:512],
                    start=(j == 0), stop=(j == 7),
                )
                nc.tensor.matmul(
                    psum_v1[:], wv_sb[:, j : j + 1], Wt[:, 512:1024],
                    start=(j == 0), stop=(j == 7),
                )
            v_sb = singles.tile([1, H2], FP32)
            nc.vector.tensor_copy(v_sb[:, 0:512], psum_v0[:])
            nc.vector.tensor_copy(v_sb[:, 512:1024], psum_v1[:])
            identity = singles.tile([128, 128], FP32)
            make_identity(nc, identity[:])
            ones_row = singles.tile([1, 128], FP32)
            nc.vector.memset(ones_row[:], 1.0)
            ones_col = singles.tile([128, 1], FP32)
            nc.vector.memset(ones_col[:], 1.0)
            vps = psum.tile([128, H2], FP32, tag="vps")
            nc.tensor.matmul(vps[:, 0:512], ones_row[:], v_sb[:, 0:512])
            nc.tensor.matmul(vps[:, 512:1024], ones_row[:], v_sb[:, 512:1024])
            v_rep = singles.tile([128, H2], FP32)
            nc.vector.tensor_copy(v_rep[:], vps[:])

            # ---- energies[s] = x[s] . v via fused multiply-reduce on DVE ----
            # Layout: partition p owns seq rows [64p, 64p+64); energies[p, c] is
            # local row 64p + c, so the DRAM store is contiguous per partition.
            x_r = x.rearrange("(p c) h -> p c h", p=128)
            dve_only = mode.startswith("dve") or mode == "both"
            variant = mode[3:] if mode.startswith("dve") else (mode[6:] if mode.startswith("stream") else "")
            if dve_only:
                xt0 = singles.tile([128, TILE_R, H2], FP32)
                nc.sync.dma_start(out=xt0[:], in_=x_r[:, 0:TILE_R, :])
            for rep in range(n_reps):
              energies = epool.tile([128, ROWS_PER_PART], FP32, tag="energies")
              for _w in range(work_mult):
                for t in range(N_TILES):
                    if mode == "both":
                        xt_l = data.tile([128, TILE_R, H2], FP32, tag="xt")
                        dma = nc.sync if dma_eng == "sync" else nc.gpsimd
                        dma.dma_start(
                            out=xt_l[:], in_=x_r[:, TILE_R * t : TILE_R * (t + 1), :]
                        )
                    if not dve_only:
                        xt = data.tile([128, TILE_R, H2], FP32, tag="xt")
                        dma = nc.sync if dma_eng == "sync" else nc.gpsimd
                        dma.dma_start(
                            out=xt[:], in_=x_r[:, TILE_R * t : TILE_R * (t + 1), :]
                        )
                    else:
                        xt = xt0
                    if mode == "dma":
                        continue
                    if variant == "5":
                        # hybrid: rows split between STT-on-DVE and
                        # mult-on-DVE + accumulate-on-ACT
                        for r in range(TILE_R):
                            c = TILE_R * t + r
                            dm1 = scratch.tile([128, 1], FP32, tag="dm1")
                            if (c % 8) < act8:
                                sc = scratch.tile([128, H2], FP32, tag="sc")
                                nc.vector.tensor_mul(sc[:], xt[:, r, :], v_rep[:])
                                nc.scalar.activation(
                                    dm1.broadcast_to(sc[:].shape),
                                    sc[:],
                                    mybir.ActivationFunctionType.Copy,
                                    accum_out=energies[:, c : c + 1],
                                )
                            else:
                                nc.vector.scalar_tensor_tensor(
                                    out=dm1.broadcast_to(xt[:, r, :].shape),
                                    in0=xt[:, r, :],
                                    scalar=1.0,
                                    in1=v_rep[:],
                                    op0=mybir.AluOpType.bypass,
                                    op1=mybir.AluOpType.mult,
                                    accum_out=energies[:, c : c + 1],
                                )
                    elif variant in ("8", "9"):
                        # STT with v operand in PSUM (frees SBUF read port)
                        for r in range(TILE_R):
                            c = TILE_R * t + r
                            if variant == "8":
                                dmx = scratch.tile([128, 1], FP32, tag="dm1")
                            else:
                                dmx = psum.tile([128, 1], FP32, tag="dm1p")
                            nc.vector.scalar_tensor_tensor(
                                out=dmx.broadcast_to(xt[:, r, :].shape),
                                in0=xt[:, r, :],
                                scalar=1.0,
                                in1=vps[:],
                                op0=mybir.AluOpType.bypass,
                                op1=mybir.AluOpType.mult,
                                accum_out=energies[:, c : c + 1],
                            )
                    elif variant == "7":
                        # STT with PSUM dummy out (keep SBUF write port for DMA)
                        for r in range(TILE_R):
                            c = TILE_R * t + r
                            dm1p = psum.tile([128, 1], FP32, tag="dm1p")
                            nc.vector.scalar_tensor_tensor(
                                out=dm1p.broadcast_to(xt[:, r, :].shape),
                                in0=xt[:, r, :],
                                scalar=1.0,
                                in1=v_rep[:],
                                op0=mybir.AluOpType.bypass,
                                op1=mybir.AluOpType.mult,
                                accum_out=energies[:, c : c + 1],
                            )
                    elif variant == "6":
                        # rows split between DVE-STT and GpSimd-STT
                        for r in range(TILE_R):
                            c = TILE_R * t + r
                            dm1 = scratch.tile([128, 1], FP32, tag="dm1")
                            eng = nc.gpsimd if (c % 4) == 3 else nc.vector
                            eng.scalar_tensor_tensor(
                                out=dm1.broadcast_to(xt[:, r, :].shape),
                                in0=xt[:, r, :],
                                scalar=1.0,
                                in1=v_rep[:],
                                op0=mybir.AluOpType.bypass,
                                op1=mybir.AluOpType.mult,
                                accum_out=energies[:, c : c + 1],
                            )
                    elif variant == "4":
                        # fused STT but discard product via stride-0 out
                        for r in range(TILE_R):
                            c = TILE_R * t + r
                            dm1 = scratch.tile([128, 1], FP32, tag="dm1")
                            nc.vector.scalar_tensor_tensor(
                                out=dm1.broadcast_to(xt[:, r, :].shape),
                                in0=xt[:, r, :],
                                scalar=1.0,
                                in1=v_rep[:],
                                op0=mybir.AluOpType.bypass,
                                op1=mybir.AluOpType.mult,
                                accum_out=energies[:, c : c + 1],
                            )
                    else:
                        for r in range(TILE_R):
                            c = TILE_R * t + r
                            sc = scratch.tile([128, H2], FP32, tag="sc")
                            nc.vector.scalar_tensor_tensor(
                                out=sc[:],
                                in0=xt[:, r, :],
                                scalar=1.0,
                                in1=v_rep[:],
                                op0=mybir.AluOpType.bypass,
                                op1=mybir.AluOpType.mult,
                                accum_out=energies[:, c : c + 1],
                            )

              if mode != "full":
                  if mode != "dma":
                      nc.sync.dma_start(
                          out=out[:].rearrange("(p c) -> p c", p=128),
                          in_=energies[:],
                      )
                  continue

              # ---- local softmax stats, AllGather 8 bytes/core of (max, sum) ----
              pmax = epool.tile([128, 1], FP32, tag="pmax")
              nc.vector.tensor_reduce(
                  out=pmax[:], in_=energies[:],
                  axis=mybir.AxisListType.X, op=mybir.AluOpType.max,
              )
              # cross-partition max via PE transpose + row reduce
              pT = psum.tile([1, 128], FP32, tag="pT")
              nc.tensor.transpose(pT[:], pmax[:], identity[:])
              pTs = epool.tile([1, 128], FP32, tag="pTs")
              nc.vector.tensor_copy(pTs[:], pT[:])
              lmax1 = epool.tile([1, 1], FP32, tag="lmax1")
              nc.vector.tensor_reduce(
                  out=lmax1[:], in_=pTs[:],
                  axis=mybir.AxisListType.X, op=mybir.AluOpType.max,
              )
              # broadcast -lmax to all partitions via ones-matmul
              nl_ps = psum.tile([128, 1], FP32, tag="nl")
              nc.tensor.matmul(nl_ps[:], ones_row[:], lmax1[:])
              nlmax = epool.tile([128, 1], FP32, tag="nlmax")
              nc.scalar.mul(nlmax[:], nl_ps[:], -1.0)
              eexp = epool.tile([128, ROWS_PER_PART], FP32, tag="eexp")
              psum1 = epool.tile([128, 1], FP32, tag="psum1")
              nc.scalar.activation(
                  eexp[:], energies[:], mybir.ActivationFunctionType.Exp,
                  bias=nlmax[:], scale=1.0, accum_out=psum1[:],
              )
              # cross-partition sum via ones-matmul
              ls_ps = psum.tile([1, 1], FP32, tag="ls")
              nc.tensor.matmul(ls_ps[:], ones_col[:], psum1[:])
              stats = epool.tile([1, 2], FP32, tag="stats")
              nc.vector.tensor_copy(stats[:, 0:1], lmax1[:])
              nc.vector.tensor_copy(stats[:, 1:2], ls_ps[:])
              ag_in = dram.tile([2], FP32, tag="ag_in")
              ag_out = dram.tile([2 * N_CORES], FP32, addr_space="Shared", tag="ag_out")
              nc.gpsimd.dma_start(out=ag_in[:].rearrange("(o c) -> o c", o=1), in_=stats[:])
              nc.gpsimd.collective_compute(
                  "AllGather",
                  mybir.AluOpType.bypass,
                  replica_groups=[list(range(N_CORES))],
                  ins=[ag_in.opt()],
                  outs=[ag_out.opt()],
              )
              # gathered stats: [8, 2] -> maxs on one partition-0 row each
              maxs = epool.tile([1, N_CORES], FP32, tag="maxs")
              sums = epool.tile([1, N_CORES], FP32, tag="sums")
              ag_r = ag_out[:].rearrange("(c k) -> k c", k=2)
              nc.gpsimd.dma_start(out=maxs[:], in_=ag_r[0:1, :])
              nc.gpsimd.dma_start(out=sums[:], in_=ag_r[1:2, :])
              gmax = epool.tile([1, 1], FP32, tag="gmax")
              nc.vector.tensor_reduce(
                  out=gmax[:], in_=maxs[:],
                  axis=mybir.AxisListType.X, op=mybir.AluOpType.max,
              )
              ngmax = epool.tile([1, 1], FP32, tag="ngmax")
              nc.scalar.mul(ngmax[:], gmax[:], -1.0)
              # gsum = sum_c lsum_c * exp(lmax_c - gmax)
              e8 = epool.tile([1, N_CORES], FP32, tag="e8")
              nc.scalar.activation(
                  e8[:], maxs[:], mybir.ActivationFunctionType.Exp,
                  bias=ngmax[:], scale=1.0,
              )
              prod = epool.tile([1, N_CORES], FP32, tag="prod")
              nc.vector.tensor_mul(prod[:], e8[:], sums[:])
              gsum = epool.tile([1, 1], FP32, tag="gsum")
              nc.vector.tensor_reduce(
                  out=gsum[:], in_=prod[:],
                  axis=mybir.AxisListType.X, op=mybir.AluOpType.add,
              )
              # k = exp(lmax - gmax) / gsum, broadcast to all partitions
              e1 = epool.tile([1, 1], FP32, tag="e1")
              nc.scalar.activation(
                  e1[:], lmax1[:], mybir.ActivationFunctionType.Exp,
                  bias=ngmax[:], scale=1.0,
              )
              invg = epool.tile([1, 1], FP32, tag="invg")
              nc.vector.reciprocal(invg[:], gsum[:])
              kfac = epool.tile([1, 1], FP32, tag="kfac")
              nc.vector.tensor_mul(kfac[:], e1[:], invg[:])
              kb_ps = psum.tile([128, 1], FP32, tag="kbp")
              nc.tensor.matmul(kb_ps[:], ones_row[:], kfac[:])
              kb = epool.tile([128, 1], FP32, tag="kb")
              nc.vector.tensor_copy(kb[:], kb_ps[:])
              outt = epool.tile([128, ROWS_PER_PART], FP32, tag="outt")
              nc.vector.tensor_scalar_mul(outt[:], eexp[:], kb[:])
              nc.gpsimd.dma_start(
                  out=out[:].rearrange("(p c) -> p c", p=128), in_=outt[:]
              )

    nc.compile()
    return nc


def _get_nc(work_mult=1, n_reps=1, mode="full", **kw):
    key = ("nc", work_mult, n_reps, mode, tuple(sorted(kw.items())))
    if key not in _nc_cache:
        _nc_cache[key] = _build_nc(work_mult, n_reps, mode, **kw)
    return _nc_cache[key]


def run(outputs, W, b, weight_vec, trace=False):
    del b  # dot(wv, b) is a constant energy offset; softmax is shift-invariant
    nc = _get_nc()
    outputs = np.ascontiguousarray(outputs, dtype=np.float32)
    W = np.ascontiguousarray(W, dtype=np.float32)
    wv = np.ascontiguousarray(weight_vec, dtype=np.float32).reshape(1, H2)
    in_maps = [
        {"x": outputs[c * LOCAL : (c + 1) * LOCAL], "W": W, "wv": wv}
        for c in range(N_CORES)
    ]
    res = run_bass_kernel_spmd(nc, in_maps, list(range(N_CORES)), trace=trace)
    attn = np.concatenate([res.results[c]["out"] for c in range(N_CORES)])
    return attn.reshape(1, 1, SEQ), res


def kernel(outputs, W, b, weight_vec):
    attn, _ = run(outputs, W, b, weight_vec)
    return attn


# revision 22
# speedup vs baseline: 82651.5393x; 77939.2385x over previous
"""Trainium2 Bass kernel for nn_Attn: attn = softmax(outputs @ W.T @ wv + b @ wv).

Math: energy[s] = dot(wv, W @ outputs[s] + b) = outputs[s] . (wv @ W) + const.
The const (wv . b) cancels in softmax, and W collapses into v = wv @ W, so the
heavy work is a memory-bound [65536, 1024] @ [1024] matvec (256 MB of reads).

Sharding: outputs split along seq across 8 cores (8192 rows / 32 MB each).
Each core computes v = wv @ W on the PE (W replicated), broadcasts v across
partitions, streams its shard through a fused multiply-reduce on the vector
engine, AllGathers the 65536 energies (256 KB), and does the global softmax
on-chip. Every core writes the full [65536] softmax; the host takes core 0's.
"""

import sys

if "/opt/trn_rl_repo" not in sys.path:
    sys.path.insert(0, "/opt/trn_rl_repo")

import numpy as np

import concourse.bacc as bacc
from concourse.masks import make_identity
import concourse.bass_isa as bass_isa
import concourse.mybir as mybir
import concourse.tile as tile
from concourse.bass_utils import run_bass_kernel_spmd

N_CORES = 8
SEQ = 65536
H2 = 1024
LOCAL = SEQ // N_CORES          # 8192 rows per core
ROWS_PER_PART = LOCAL // 128    # 64 seq rows per SBUF partition
TILE_R = 4                      # seq rows per partition per data tile (2 MB tiles)
N_TILES = ROWS_PER_PART // TILE_R

FP32 = mybir.dt.float32

_nc_cache = {}


def _build_nc(work_mult=1, n_reps=1, mode="full", data_bufs=12, tile_r=2,
              scratch_bufs=4, dma_eng="sync", act8=5, shard_v=0):
    TILE_R = tile_r
    N_TILES = ROWS_PER_PART // TILE_R
    nc = bacc.Bacc("TRN2", target_bir_lowering=False)
    x = nc.dram_tensor("x", [LOCAL, H2], FP32, kind="ExternalInput")
    HC = H2 // N_CORES  # 128 columns of W per core when shard_v
    if shard_v:
        W = nc.dram_tensor("Wc", [H2, HC], FP32, kind="ExternalInput")
    else:
        W = nc.dram_tensor("W", [H2, H2], FP32, kind="ExternalInput")
    wv = nc.dram_tensor("wv", [1, H2], FP32, kind="ExternalInput")
    out = nc.dram_tensor("out", [LOCAL], FP32, kind="ExternalOutput")

    with tile.TileContext(nc) as tc:
        with (
            tc.tile_pool(name="singles", bufs=1) as singles,
            tc.tile_pool(name="wpool", bufs=2) as wpool,
            tc.tile_pool(name="data", bufs=data_bufs) as data,
            tc.tile_pool(name="scratch", bufs=scratch_bufs) as scratch,
            tc.tile_pool(name="epool", bufs=2) as epool,
            tc.tile_pool(name="psum", bufs=1, space="PSUM") as psum,
            tc.tile_pool(name="dram", bufs=1, space="DRAM") as dram,
        ):
            # ---- v = wv @ W on the PE; lhsT chunks are wv with o on partitions ----
            wv_sb = singles.tile([128, 8], FP32)
            nc.sync.dma_start(
                out=wv_sb[:], in_=wv[:].rearrange("1 (j p) -> p j", p=128)
            )
            v_sb = singles.tile([1, H2], FP32)
            if shard_v:
                # each core computes its 128-col slice of v, AllGather the rest
                psum_vc = psum.tile([1, HC], FP32, tag="psv0")
                for j in range(8):
                    Wt = wpool.tile([128, HC], FP32)
                    nc.sync.dma_start(out=Wt[:], in_=W[128 * j : 128 * (j + 1), :])
                    nc.tensor.matmul(
                        psum_vc[:], wv_sb[:, j : j + 1], Wt[:],
                        start=(j == 0), stop=(j == 7),
                    )
                vc_sb = singles.tile([1, HC], FP32)
                nc.vector.tensor_copy(vc_sb[:], psum_vc[:])
                vag_in = dram.tile([HC], FP32, tag="vag_in")
                vag_out = dram.tile([H2], FP32, addr_space="Shared", tag="vag_out")
                nc.gpsimd.dma_start(
                    out=vag_in[:].rearrange("(o c) -> o c", o=1), in_=vc_sb[:]
                )
                nc.gpsimd.collective_compute(
                    "AllGather",
                    mybir.AluOpType.bypass,
                    replica_groups=[list(range(N_CORES))],
                    ins=[vag_in.opt()],
                    outs=[vag_out.opt()],
                )
                nc.gpsimd.dma_start(
                    out=v_sb[:], in_=vag_out[:].rearrange("(o c) -> o c", o=1)
                )
            else:
                psum_v0 = psum.tile([1, 512], FP32, tag="psv0")
                psum_v1 = psum.tile([1, 512], FP32, tag="psv1")
                for j in range(8):
                    Wt = wpool.tile([128, H2], FP32)
                    nc.sync.dma_start(out=Wt[:], in_=W[128 * j : 128 * (j + 1), :])
                    nc.tensor.matmul(
                        psum_v0[:], wv_sb[:, j : j + 1], Wt[:, 0:512],
                        start=(j == 0), stop=(j == 7),
                    )
                    nc.tensor.matmul(
                        psum_v1[:], wv_sb[:, j : j + 1], Wt[:, 512:1024],
                        start=(j == 0), stop=(j == 7),
                    )
                nc.vector.tensor_copy(v_sb[:, 0:512], psum_v0[:])
                nc.vector.tensor_copy(v_sb[:, 512:1024], psum_v1[:])
            identity = singles.tile([128, 128], FP32)
            make_identity(nc, identity[:])
            ones_row = singles.tile([1, 128], FP32)
            nc.vector.memset(ones_row[:], 1.0)
            ones_col = singles.tile([128, 1], FP32)
            nc.vector.memset(ones_col[:], 1.0)
            vps = psum.tile([128, H2], FP32, tag="vps")
            nc.tensor.matmul(vps[:, 0:512], ones_row[:], v_sb[:, 0:512])
            nc.tensor.matmul(vps[:, 512:1024], ones_row[:], v_sb[:, 512:1024])
            v_rep = singles.tile([128, H2], FP32)
            nc.vector.tensor_copy(v_rep[:], vps[:])

            # ---- energies[s] = x[s] . v via fused multiply-reduce on DVE ----
            # Layout: partition p owns seq rows [64p, 64p+64); energies[p, c] is
            # local row 64p + c, so the DRAM store is contiguous per partition.
            x_r = x.rearrange("(p c) h -> p c h", p=128)
            dve_only = mode.startswith("dve") or mode == "both"
            variant = mode[3:] if mode.startswith("dve") else (mode[6:] if mode.startswith("stream") else "")
            if dve_only:
                xt0 = singles.tile([128, TILE_R, H2], FP32)
                nc.sync.dma_start(out=xt0[:], in_=x_r[:, 0:TILE_R, :])
            for rep in range(n_reps):
              energies = epool.tile([128, ROWS_PER_PART], FP32, tag="energies")
              for _w in range(work_mult):
                for t in range(N_TILES):
                    if mode == "both":
                        xt_l = data.tile([128, TILE_R, H2], FP32, tag="xt")
                        dma = nc.sync if dma_eng == "sync" else nc.gpsimd
                        dma.dma_start(
                            out=xt_l[:], in_=x_r[:, TILE_R * t : TILE_R * (t + 1), :]
                        )
                    if not dve_only:
                        xt = data.tile([128, TILE_R, H2], FP32, tag="xt")
                        dma = nc.sync if dma_eng == "sync" else nc.gpsimd
                        dma.dma_start(
                            out=xt[:], in_=x_r[:, TILE_R * t : TILE_R * (t + 1), :]
                        )
                    else:
                        xt = xt0
                    if mode == "dma":
                        continue
                    if variant == "5":
                        # hybrid: rows split between STT-on-DVE and
                        # mult-on-DVE + accumulate-on-ACT
                        for r in range(TILE_R):
                            c = TILE_R * t + r
                            dm1 = scratch.tile([128, 1], FP32, tag="dm1")
                            if (c % 8) < act8:
                                sc = scratch.tile([128, H2], FP32, tag="sc")
                                nc.vector.tensor_mul(sc[:], xt[:, r, :], v_rep[:])
                                nc.scalar.activation(
                                    dm1.broadcast_to(sc[:].shape),
                                    sc[:],
                                    mybir.ActivationFunctionType.Copy,
                                    accum_out=energies[:, c : c + 1],
                                )
                            else:
                                nc.vector.scalar_tensor_tensor(
                                    out=dm1.broadcast_to(xt[:, r, :].shape),
                                    in0=xt[:, r, :],
                                    scalar=1.0,
                                    in1=v_rep[:],
                                    op0=mybir.AluOpType.bypass,
                                    op1=mybir.AluOpType.mult,
                                    accum_out=energies[:, c : c + 1],
                                )
                    elif variant in ("8", "9"):
                        # STT with v operand in PSUM (frees SBUF read port)
                        for r in range(TILE_R):
                            c = TILE_R * t + r
                            if variant == "8":
                                dmx = scratch.tile([128, 1], FP32, tag="dm1")
                            else:
                                dmx = psum.tile([128, 1], FP32, tag="dm1p")
                            nc.vector.scalar_tensor_tensor(
                                out=dmx.broadcast_to(xt[:, r, :].shape),
                                in0=xt[:, r, :],
                                scalar=1.0,
                                in1=vps[:],
                                op0=mybir.AluOpType.bypass,
                                op1=mybir.AluOpType.mult,
                                accum_out=energies[:, c : c + 1],
                            )
                    elif variant == "7":
                        # STT with PSUM dummy out (keep SBUF write port for DMA)
                        for r in range(TILE_R):
                            c = TILE_R * t + r
                            dm1p = psum.tile([128, 1], FP32, tag="dm1p")
                            nc.vector.scalar_tensor_tensor(
                                out=dm1p.broadcast_to(xt[:, r, :].shape),
                                in0=xt[:, r, :],
                                scalar=1.0,
                                in1=v_rep[:],
                                op0=mybir.AluOpType.bypass,
                                op1=mybir.AluOpType.mult,
                                accum_out=energies[:, c : c + 1],
                            )
                    elif variant == "6":
                        # rows split between DVE-STT and GpSimd-STT
                        for r in range(TILE_R):
                            c = TILE_R * t + r
                            dm1 = scratch.tile([128, 1], FP32, tag="dm1")
                            eng = nc.gpsimd if (c % 4) == 3 else nc.vector
                            eng.scalar_tensor_tensor(
                                out=dm1.broadcast_to(xt[:, r, :].shape),
                                in0=xt[:, r, :],
                                scalar=1.0,
                                in1=v_rep[:],
                                op0=mybir.AluOpType.bypass,
                                op1=mybir.AluOpType.mult,
                                accum_out=energies[:, c : c + 1],
                            )
                    elif variant == "4":
                        # fused STT but discard product via stride-0 out
                        for r in range(TILE_R):
                            c = TILE_R * t + r
                            dm1 = scratch.tile([128, 1], FP32, tag="dm1")
                            nc.vector.scalar_tensor_tensor(
                                out=dm1.broadcast_to(xt[:, r, :].shape),
                                in0=xt[:, r, :],
                                scalar=1.0,
                                in1=v_rep[:],
                                op0=mybir.AluOpType.bypass,
                                op1=mybir.AluOpType.mult,
                                accum_out=energies[:, c : c + 1],
                            )
                    else:
                        for r in range(TILE_R):
                            c = TILE_R * t + r
                            sc = scratch.tile([128, H2], FP32, tag="sc")
                            nc.vector.scalar_tensor_tensor(
                                out=sc[:],
                                in0=xt[:, r, :],
                                scalar=1.0,
                                in1=v_rep[:],
                                op0=mybir.AluOpType.bypass,
                                op1=mybir.AluOpType.mult,
                                accum_out=energies[:, c : c + 1],
                            )

              if mode != "full":
                  if mode != "dma":
                      nc.sync.dma_start(
                          out=out[:].rearrange("(p c) -> p c", p=128),
                          in_=energies[:],
                      )
                  continue

              # ---- local softmax stats, AllGather 8 bytes/core of (max, sum) ----
              pmax = epool.tile([128, 1], FP32, tag="pmax")
              nc.vector.tensor_reduce(
                  out=pmax[:], in_=energies[:],
                  axis=mybir.AxisListType.X, op=mybir.AluOpType.max,
              )
              # cross-partition max via PE transpose + row reduce
              pT = psum.tile([1, 128], FP32, tag="pT")
              nc.tensor.transpose(pT[:], pmax[:], identity[:])
              pTs = epool.tile([1, 128], FP32, tag="pTs")
              nc.vector.tensor_copy(pTs[:], pT[:])
              lmax1 = epool.tile([1, 1], FP32, tag="lmax1")
              nc.vector.tensor_reduce(
                  out=lmax1[:], in_=pTs[:],
                  axis=mybir.AxisListType.X, op=mybir.AluOpType.max,
              )
              # broadcast -lmax to all partitions via ones-matmul
              nl_ps = psum.tile([128, 1], FP32, tag="nl")
              nc.tensor.matmul(nl_ps[:], ones_row[:], lmax1[:])
              nlmax = epool.tile([128, 1], FP32, tag="nlmax")
              nc.scalar.mul(nlmax[:], nl_ps[:], -1.0)
              eexp = epool.tile([128, ROWS_PER_PART], FP32, tag="eexp")
              psum1 = epool.tile([128, 1], FP32, tag="psum1")
              nc.scalar.activation(
                  eexp[:], energies[:], mybir.ActivationFunctionType.Exp,
                  bias=nlmax[:], scale=1.0, accum_out=psum1[:],
              )
              # cross-partition sum via ones-matmul
              ls_ps = psum.tile([1, 1], FP32, tag="ls")
              nc.tensor.matmul(ls_ps[:], ones_col[:], psum1[:])
              stats = epool.tile([1, 2], FP32, tag="stats")
              nc.vector.tensor_copy(stats[:, 0:1], lmax1[:])
              nc.vector.tensor_copy(stats[:, 1:2], ls_ps[:])
              ag_in = dram.tile([2], FP32, tag="ag_in")
              ag_out = dram.tile([2 * N_CORES], FP32, addr_space="Shared", tag="ag_out")
              nc.gpsimd.dma_start(out=ag_in[:].rearrange("(o c) -> o c", o=1), in_=stats[:])
              nc.gpsimd.collective_compute(
                  "AllGather",
                  mybir.AluOpType.bypass,
                  replica_groups=[list(range(N_CORES))],
                  ins=[ag_in.opt()],
                  outs=[ag_out.opt()],
              )
              # gathered stats: [8, 2] -> maxs on one partition-0 row each
              maxs = epool.tile([1, N_CORES], FP32, tag="maxs")
              sums = epool.tile([1, N_CORES], FP32, tag="sums")
              ag_r = ag_out[:].rearrange("(c k) -> k c", k=2)
              nc.gpsimd.dma_start(out=maxs[:], in_=ag_r[0:1, :])
              nc.gpsimd.dma_start(out=sums[:], in_=ag_r[1:2, :])
              gmax = epool.tile([1, 1], FP32, tag="gmax")
              nc.vector.tensor_reduce(
                  out=gmax[:], in_=maxs[:],
                  axis=mybir.AxisListType.X, op=mybir.AluOpType.max,
              )
              ngmax = epool.tile([1, 1], FP32, tag="ngmax")
              nc.scalar.mul(ngmax[:], gmax[:], -1.0)
              # gsum = sum_c lsum_c * exp(lmax_c - gmax)
              e8 = epool.tile([1, N_CORES], FP32, tag="e8")
              nc.scalar.activation(
                  e8[:], maxs[:], mybir.ActivationFunctionType.Exp,
                  bias=ngmax[:], scale=1.0,
              )
              prod = epool.tile([1, N_CORES], FP32, tag="prod")
              nc.vector.tensor_mul(prod[:], e8[:], sums[:])
              gsum = epool.tile([1, 1], FP32, tag="gsum")
              nc.vector.tensor_reduce(
                  out=gsum[:], in_=prod[:],
                  axis=mybir.AxisListType.X, op=mybir.AluOpType.add,
              )
              # k = exp(lmax - gmax) / gsum, broadcast to all partitions
              e1 = epool.tile([1, 1], FP32, tag="e1")
              nc.scalar.activation(
                  e1[:], lmax1[:], mybir.ActivationFunctionType.Exp,
                  bias=ngmax[:], scale=1.0,
              )
              invg = epool.tile([1, 1], FP32, tag="invg")
              nc.vector.reciprocal(invg[:], gsum[:])
              kfac = epool.tile([1, 1], FP32, tag="kfac")
              nc.vector.tensor_mul(kfac[:], e1[:], invg[:])
              kb_ps = psum.tile([128, 1], FP32, tag="kbp")
              nc.tensor.matmul(kb_ps[:], ones_row[:], kfac[:])
              kb = epool.tile([128, 1], FP32, tag="kb")
              nc.vector.tensor_copy(kb[:], kb_ps[:])
              outt = epool.tile([128, ROWS_PER_PART], FP32, tag="outt")
              nc.vector.tensor_scalar_mul(outt[:], eexp[:], kb[:])
              nc.gpsimd.dma_start(
                  out=out[:].rearrange("(p c) -> p c", p=128), in_=outt[:]
              )

    nc.compile()
    return nc


def _get_nc(work_mult=1, n_reps=1, mode="full", **kw):
    key = ("nc", work_mult, n_reps, mode, tuple(sorted(kw.items())))
    if key not in _nc_cache:
        _nc_cache[key] = _build_nc(work_mult, n_reps, mode, **kw)
    return _nc_cache[key]


SHARD_V = 1


def run(outputs, W, b, weight_vec, trace=False):
    del b  # dot(wv, b) is a constant energy offset; softmax is shift-invariant
    nc = _get_nc(shard_v=SHARD_V)
    outputs = np.ascontiguousarray(outputs, dtype=np.float32)
    W = np.ascontiguousarray(W, dtype=np.float32)
    wv = np.ascontiguousarray(weight_vec, dtype=np.float32).reshape(1, H2)
    HC = H2 // N_CORES
    if SHARD_V:
        in_maps = [
            {
                "x": outputs[c * LOCAL : (c + 1) * LOCAL],
                "Wc": np.ascontiguousarray(W[:, c * HC : (c + 1) * HC]),
                "wv": wv,
            }
            for c in range(N_CORES)
        ]
    else:
        in_maps = [
            {"x": outputs[c * LOCAL : (c + 1) * LOCAL], "W": W, "wv": wv}
            for c in range(N_CORES)
        ]
    res = run_bass_kernel_spmd(nc, in_maps, list(range(N_CORES)), trace=trace)
    attn = np.concatenate([res.results[c]["out"] for c in range(N_CORES)])
    return attn.reshape(1, 1, SEQ), res


def kernel(outputs, W, b, weight_vec):
    attn, _ = run(outputs, W, b, weight_vec)
    return attn
